# revision 1
# baseline (speedup 1.0000x reference)
"""Causal self-attention with RoPE + XSA (self-value subtraction), Trainium2.

Sharding: tensor-parallel over heads. 16 heads / 8 cores = 2 heads per core.
Each core computes QKV for its 2 heads (full batch), flash-style causal
attention in S^T layout (k on partitions, q on free dim), and a partial
output projection over its 128 feature columns. Host sums the 8 partials.

All matmuls run in float32r (full-rate fp32, ~1.6e-4 rel err on HW).

Layout notes (per core, per batch b):
  A_q, A_k : [128, 2048]  q^T/k^T, rows 0..63 = head h0 dims, 64..127 = h1
  VT       : [128, 2048]  v^T, same row layout (no RoPE)
  vext     : [128, 16, 2, 65] v tok-major per 128-tok tile per head + ones col
  attention: S^T[k, q] = matmul(lhsT=K^T[d, kc*128:], rhs=Q^T[d, qj*512:])
             P = exp(S^T / 8); V-matmul out^T[d(+denom), q] with ones column
  XSA      : strict mask (k<q) zeroes diag+future; diag exp added to the
             denominator via a tiny K=2 matmul from elementwise q.k products.
"""

import sys

if "/opt/trn_rl_repo" not in sys.path:
    sys.path.insert(0, "/opt/trn_rl_repo")

import numpy as np

B, T, D, H = 4, 2048, 1024, 16
DH = D // H  # 64
HALF = DH // 2  # 32
NCORES = 8
HPC = H // NCORES  # 2 heads per core
N = B * T  # 8192
QC = 512  # q chunk
KC = 128  # k chunk
NQJ = T // QC  # 4 q chunks per (b)
FC = D // 128  # 8 feature chunks
TC = T // QC  # 4 tok chunks per b


def _build():
    import concourse.bass as bass
    import concourse.mybir as mybir
    import concourse.tile as tile
    from concourse import bacc

    F32 = mybir.dt.float32
    F32R = mybir.dt.float32r
    AF = mybir.ActivationFunctionType
    ALU = mybir.AluOpType
    ds, ts = bass.ds, bass.ts

    nc = bacc.Bacc("TRN2")

    xT_d = nc.dram_tensor("xT", (D, N), F32, kind="ExternalInput")
    cosr_d = nc.dram_tensor("cosr", (128, T), F32, kind="ExternalInput")
    sinr_d = nc.dram_tensor("sinr", (128, T), F32, kind="ExternalInput")
    wqk_d = nc.dram_tensor("wqkT", (D, 384), F32, kind="ExternalInput")
    wp_d = nc.dram_tensor("wpT", (128, D), F32, kind="ExternalInput")
    esel_d = nc.dram_tensor("esel", (128, 2), F32, kind="ExternalInput")
    lsel_d = nc.dram_tensor("lsel", (2, 2, 65), F32, kind="ExternalInput")
    strictu_d = nc.dram_tensor("strictu", (128, 128), F32, kind="ExternalInput")
    ident_d = nc.dram_tensor("ident", (128, 128), F32, kind="ExternalInput")
    ones_d = nc.dram_tensor("ones", (128, 64), F32, kind="ExternalInput")
    out_d = nc.dram_tensor("outp", (N, D), F32, kind="ExternalOutput")

    with tile.TileContext(nc) as tc:
        with (
            tc.tile_pool(name="p1", bufs=1) as p1,
            tc.tile_pool(name="p2", bufs=2) as p2,
            tc.tile_pool(name="pxt", bufs=10) as pxt,
            tc.tile_pool(name="ppt", bufs=4) as ppt,
            tc.tile_pool(name="psA", bufs=4, space="PSUM") as psA,
            tc.tile_pool(name="psO", bufs=4, space="PSUM") as psO,
        ):
            # --- persistent weights / constants ---
            wqk_sb = p1.tile([128, FC, 384], F32R, tag="wqk")
            nc.sync.dma_start(
                wqk_sb[:], wqk_d[:].rearrange("(o p) m -> p o m", p=128).bitcast(F32R)
            )
            wp_sb = p1.tile([128, D], F32R, tag="wp")
            nc.sync.dma_start(wp_sb[:], wp_d[:].bitcast(F32R))
            cosr = p1.tile([128, T], F32, tag="cosr")
            nc.sync.dma_start(cosr[:], cosr_d[:])
            sinr = p1.tile([128, T], F32, tag="sinr")
            nc.sync.dma_start(sinr[:], sinr_d[:])
            esel_sb = p1.tile([128, 2], F32R, tag="esel")
            nc.sync.dma_start(esel_sb[:], esel_d[:].bitcast(F32R))
            lsel_sb = p1.tile([2, 2, 65], F32R, tag="lsel")
            nc.sync.dma_start(lsel_sb[:], lsel_d[:].bitcast(F32R))
            strictu = p1.tile([128, 128], F32R, tag="strictu")
            nc.sync.dma_start(strictu[:], strictu_d[:].bitcast(F32R))
            ident = p1.tile([128, 128], F32, tag="ident")
            nc.sync.dma_start(ident[:], ident_d[:])
            onesb = p1.tile([65, 64], F32R, tag="onesb")
            nc.sync.dma_start(onesb[64:65, :], ones_d[0:1, :].bitcast(F32R))

            def emit_qkv(b):
                tok0 = b * T
                A_q = p2.tile([128, T], F32R, tag="A_q", name=f"A_q{b}")
                A_k = p2.tile([128, T], F32R, tag="A_k", name=f"A_k{b}")
                VT = p2.tile([128, T], F32, tag="VT", name=f"VT{b}")
                qkp = p2.tile([128, T], F32R, tag="qkp", name=f"qkp{b}")
                vext = p2.tile([128, T // 128, 2, 65], F32R, tag="vext",
                               name=f"vext{b}")
                nc.sync.dma_start(
                    vext[:, :, :, 64],
                    ones_d[:, 0:32].rearrange("p (a c) -> p a c", c=2).bitcast(F32R),
                )
                dsts = [A_q, A_k, VT]
                for tci in range(TC):
                    tcs = ds(tci * QC, QC)
                    xts = []
                    for fc in range(FC):
                        xt = pxt.tile([128, QC], F32R, tag="xt", name=f"xt{fc}")
                        nc.sync.dma_start(
                            xt[:],
                            xT_d[ts(fc, 128), ds(tok0 + tci * QC, QC)].bitcast(F32R),
                        )
                        xts.append(xt)
                    for mi in range(3):
                        pq = psA.tile([128, QC], F32, tag="pst", name=f"pq{mi}")
                        for fc in range(FC):
                            nc.tensor.matmul(
                                pq[:],
                                wqk_sb[:, fc, ts(mi, 128)],
                                xts[fc][:],
                                start=(fc == 0),
                                stop=(fc == FC - 1),
                            )
                        nc.vector.tensor_copy(dsts[mi][:, tcs], pq[:])
                    # RoPE on this token chunk
                    for A in (A_q, A_k):
                        Bt = p2.tile([128, QC], F32R, tag="Bt")
                        nc.sync.dma_start(Bt[0:32, :], A[32:64, tcs])
                        nc.sync.dma_start(Bt[32:64, :], A[0:32, tcs])
                        nc.sync.dma_start(Bt[64:96, :], A[96:128, tcs])
                        nc.sync.dma_start(Bt[96:128, :], A[64:96, tcs])
                        nc.vector.tensor_tensor(A[:, tcs], A[:, tcs],
                                                cosr[:, tcs], ALU.mult)
                        nc.vector.tensor_tensor(Bt[:], Bt[:], sinr[:, tcs], ALU.mult)
                        nc.vector.tensor_tensor(A[:, tcs], A[:, tcs], Bt[:], ALU.add)
                    nc.vector.tensor_tensor(qkp[:, tcs], A_q[:, tcs], A_k[:, tcs],
                                            ALU.mult)
                    # V token-major via PE transpose for this chunk
                    for tt in range(4 * tci, 4 * tci + 4):
                        ptr = psA.tile([128, 128], F32, tag="pst", name="ptr")
                        nc.tensor.transpose(ptr[:], VT[:, ts(tt, 128)], ident[:])
                        nc.vector.tensor_copy(vext[:, tt, 0, 0:64], ptr[:, 0:64])
                        nc.vector.tensor_copy(vext[:, tt, 1, 0:64], ptr[:, 64:128])
                return b, A_q, A_k, qkp, vext

            def emit_attn(st):
                b, A_q, A_k, qkp, vext = st
                outT = p2.tile([128, T], F32R, tag="outT", name=f"outT{b}")
                oT1 = p2.tile([64, T], F32R, tag="oT1", name=f"oT1{b}")
                for qj in range(NQJ):
                    q0 = qj * QC
                    pd = psA.tile([2, QC], F32, tag="pst", name="pd")
                    nc.tensor.matmul(
                        pd[:], esel_sb[:], qkp[:, ds(q0, QC)], start=True, stop=True
                    )
                    de = p2.tile([2, QC], F32R, tag="de")
                    nc.scalar.activation(de[:], pd[:], AF.Exp, scale=0.125)

                    po = [
                        psO.tile([65, QC], F32, tag="po65", name=f"po{h}")
                        for h in range(2)
                    ]
                    nkc = 4 * qj + 4
                    for kc in range(nkc):
                        o = kc - 4 * qj
                        c0 = 128 * o if o > 0 else 0
                        psts = []
                        for h in range(2):
                            r0 = 64 * h
                            pst = psA.tile([128, QC], F32, tag="pst",
                                           name=f"pst{h}")
                            nc.tensor.matmul(
                                pst[:, c0:QC],
                                A_k[r0 : r0 + 64, ts(kc, 128)],
                                A_q[r0 : r0 + 64, ds(q0 + c0, QC - c0)],
                                start=True,
                                stop=True,
                            )
                            psts.append(pst)
                        for h in range(2):
                            pt = ppt.tile([128, QC], F32R, tag="pt", name=f"pt{h}")
                            nc.scalar.activation(
                                pt[:, c0:QC], psts[h][:, c0:QC], AF.Exp, scale=0.125
                            )
                            if o >= 0:
                                nc.vector.tensor_tensor(
                                    pt[:, ds(c0, 128)],
                                    pt[:, ds(c0, 128)],
                                    strictu[:],
                                    ALU.mult,
                                )
                            nc.tensor.matmul(
                                po[h][:, c0:QC],
                                vext[:, kc, h, :],
                                pt[:, c0:QC],
                                start=(kc == 0),
                                stop=False,
                            )
                    for h in range(2):
                        nc.tensor.matmul(
                            po[h][:], lsel_sb[:, h, :], de[:], start=False, stop=True
                        )
                        dnr = p2.tile([65, QC], F32R, tag="dnr")
                        nc.scalar.copy(dnr[64:65, :], po[h][64:65, :])
                        pb = psA.tile([64, QC], F32, tag="pst", name="pb")
                        nc.tensor.matmul(
                            pb[:], onesb[64:65, :], dnr[64:65, :],
                            start=True, stop=True,
                        )
                        bc = p2.tile([64, QC], F32, tag="bc")
                        nc.vector.reciprocal_approx_fast(bc[:], pb[:])
                        dst = outT[0:64, ds(q0, QC)] if h == 0 else oT1[:, ds(q0, QC)]
                        nc.vector.tensor_tensor(dst, po[h][0:64, :], bc[:], ALU.mult)

                nc.sync.dma_start(outT[64:128, :], oT1[:])
                return outT

            def emit_proj(b, outT):
                tok0 = b * T
                for mt in range(T // 128):
                    for nj in range(2):
                        pp = psA.tile([128, 512], F32, tag="pst", name="pp")
                        nc.tensor.matmul(
                            pp[:],
                            outT[:, ts(mt, 128)],
                            wp_sb[:, ts(nj, 512)],
                            start=True,
                            stop=True,
                        )
                        po_sb = p2.tile([128, 512], F32, tag="po_sb")
                        nc.vector.tensor_copy(po_sb[:], pp[:])
                        nc.sync.dma_start(
                            out_d[ds(tok0 + mt * 128, 128), ts(nj, 512)], po_sb[:]
                        )

            sts = emit_qkv(0)
            for b in range(B):
                cur = sts
                if b + 1 < B:
                    sts = emit_qkv(b + 1)
                outT = emit_attn(cur)
                emit_proj(b, outT)

    nc.finalize()
    return nc


def _host_inputs(x, cos, sin, W_qkv, W_proj):
    """Build per-core input maps."""
    x = np.asarray(x, dtype=np.float32)
    cos = np.asarray(cos, dtype=np.float32)
    sin = np.asarray(sin, dtype=np.float32)
    W_qkv = np.asarray(W_qkv, dtype=np.float32)
    W_proj = np.asarray(W_proj, dtype=np.float32)

    xT = np.ascontiguousarray(x.reshape(N, D).T)  # [D, N]
    cosT = np.ascontiguousarray(cos[0, 0].T)  # [32, T]
    sinT = np.ascontiguousarray(sin[0, 0].T)
    cosr = np.tile(cosT, (4, 1))  # [128, T]
    sinr = np.concatenate([-sinT, sinT, -sinT, sinT], axis=0)  # [128, T]

    esel = np.zeros((128, 2), np.float32)
    esel[0:64, 0] = 1.0
    esel[64:128, 1] = 1.0
    lsel = np.zeros((2, 2, 65), np.float32)
    lsel[0, 0, 64] = 1.0  # head 0: row 0 -> out row 64
    lsel[1, 1, 64] = 1.0
    strictu = np.triu(np.ones((128, 128), np.float32), 1)  # 1 iff k < q
    ident = np.eye(128, dtype=np.float32)
    ones = np.ones((128, 64), np.float32)

    in_maps = []
    for c in range(NCORES):
        h0, h1 = 2 * c, 2 * c + 1
        cols = []
        for base in (0, D, 2 * D):  # q, k, v row blocks of W_qkv
            cols.append(W_qkv[base + 64 * h0 : base + 64 * h0 + 64])
            cols.append(W_qkv[base + 64 * h1 : base + 64 * h1 + 64])
        wqkT = np.ascontiguousarray(np.concatenate(cols, axis=0).T)  # [D, 384]
        wpT = np.ascontiguousarray(W_proj[:, 128 * c : 128 * c + 128].T)  # [128, D]
        in_maps.append(
            {
                "xT": xT,
                "cosr": cosr,
                "sinr": sinr,
                "wqkT": wqkT,
                "wpT": wpT,
                "esel": esel,
                "lsel": lsel,
                "strictu": strictu,
                "ident": ident,
                "ones": ones,
            }
        )
    return in_maps


_NC_CACHE = {}


def _get_nc():
    if "nc" not in _NC_CACHE:
        _NC_CACHE["nc"] = _build()
    return _NC_CACHE["nc"]


def kernel(x, cos, sin, W_qkv, W_proj, _trace=False, _trace_cores=None):
    from concourse import bass_utils

    nc = _get_nc()
    in_maps = _host_inputs(x, cos, sin, W_qkv, W_proj)
    res = bass_utils.run_bass_kernel_spmd(
        nc,
        in_maps,
        core_ids=list(range(NCORES)),
        trace=_trace,
        trace_cores=_trace_cores,
    )
    out = np.zeros((N, D), np.float64)
    for r in res.results:
        out += r["outp"].astype(np.float64)
    ret = out.astype(np.float32).reshape(B, T, D)
    kernel.last_results = res
    return ret



# revision 5
# speedup vs baseline: 1.8680x; 1.8680x over previous
"""Causal self-attention with RoPE + XSA (self-value subtraction), Trainium2.

Sharding: (batch x head-half). 8 cores = 4 batches x 2 groups of 8 heads.
Each core: QKV for its batch/head-group, flash-style causal attention in
S^T layout (k on partitions, q on free dim), partial output projection over
its 512 feature columns. Host sums 2 partials per batch.

vs the v1 kernel (890us): per-core DMA cut 87MB -> ~21MB, attention
operands in bf16 (full-rate PE even at small N, 2-4x DVE modes), v built
token-major directly from the QKV matmul (no PE transposes), exp-only on
the ACT engine (no table thrash), PSUM->SBUF copies on DVE, partition
broadcast on GpSimd, and fine-grained emission interleaving: QKV(tc+1) and
proj(qj-1) matmuls are woven between attention S/V matmuls so the PE never
idles waiting on ACT exp results (keeps the PE at the 2.4GHz pstate).

Layout notes (per core):
  A_q, A_k : [128, 4, 2048] bf16  q^T/k^T; tile ti rows 0..63 = head 2ti,
             64..127 = head 2ti+1 (dims within head).
  v_sb     : [128, 16, 8, 65] bf16  v token-major per 128-tok chunk per
             head + ones col (col 64) for the softmax denominator row.
  attention: S^T[k, q] = matmul(lhsT=K^T[d, kc*128:], rhs=Q^T[d, qj*512:])
             P = exp(S^T/8) -> bf16; V-matmul gives out^T[d(+denom), q].
  XSA      : strict mask (k<q) zeroes diag+future; diag exp added to the
             denominator via a tiny K=2 matmul from elementwise q.k products.
"""

import sys

if "/opt/trn_rl_repo" not in sys.path:
    sys.path.insert(0, "/opt/trn_rl_repo")

import numpy as np

B, T, D, H = 4, 2048, 1024, 16
DH = D // H  # 64
HALF = DH // 2  # 32
NCORES = 8
HPC = 8  # heads per core
QC = 512  # q chunk
KC = 128  # k chunk
NQJ = T // QC  # 4 q chunks
FC = D // 128  # 8 input-feature chunks


def _build():
    import concourse.bass as bass
    import concourse.mybir as mybir
    import concourse.tile as tile
    from concourse import bacc

    F32 = mybir.dt.float32
    F32R = mybir.dt.float32r
    BF16 = mybir.dt.bfloat16
    AF = mybir.ActivationFunctionType
    ALU = mybir.AluOpType
    ds, ts = bass.ds, bass.ts

    nc = bacc.Bacc("TRN2")

    xT_d = nc.dram_tensor("xT", (D, T), BF16, kind="ExternalInput")
    wq_d = nc.dram_tensor("wqT", (D, 512), BF16, kind="ExternalInput")
    wk_d = nc.dram_tensor("wkT", (D, 512), BF16, kind="ExternalInput")
    wv_d = nc.dram_tensor("wvT", (D, 512), BF16, kind="ExternalInput")
    wp_d = nc.dram_tensor("wpT", (512, D), BF16, kind="ExternalInput")
    cosr_d = nc.dram_tensor("cosr", (128, T), BF16, kind="ExternalInput")
    sinr_d = nc.dram_tensor("sinr", (128, T), BF16, kind="ExternalInput")
    esel_d = nc.dram_tensor("esel", (128, 2), BF16, kind="ExternalInput")
    strictu_d = nc.dram_tensor("strictu", (128, 128), BF16, kind="ExternalInput")
    lsel_d = nc.dram_tensor("lsel", (2, 2, 65), F32, kind="ExternalInput")
    out_d = nc.dram_tensor("outp", (T, D), F32, kind="ExternalOutput")

    with tile.TileContext(nc) as tc:
        with (
            tc.tile_pool(name="p1", bufs=1) as p1,
            tc.tile_pool(name="pxt", bufs=2) as pxt,
            tc.tile_pool(name="ppt", bufs=4) as ppt,
            tc.tile_pool(name="pbt", bufs=2) as pbt,
            tc.tile_pool(name="psc", bufs=2) as psc,
            tc.tile_pool(name="ps", bufs=1, space="PSUM") as ps,
        ):
            # --- persistent weights / constants ---
            # weights go on the ACT HWDGE queue so the SP queue serves the
            # first x-chunk loads immediately (shorter prologue)
            wq_sb = p1.tile([128, FC, 512], BF16, tag="wq")
            nc.scalar.dma_start(wq_sb[:], wq_d[:].rearrange("(o p) m -> p o m", p=128))
            wk_sb = p1.tile([128, FC, 512], BF16, tag="wk")
            nc.scalar.dma_start(wk_sb[:], wk_d[:].rearrange("(o p) m -> p o m", p=128))
            wv_sb = p1.tile([128, FC, 512], BF16, tag="wv")
            nc.scalar.dma_start(wv_sb[:], wv_d[:].rearrange("(o p) m -> p o m", p=128))
            wp_sb = p1.tile([128, 4, D], BF16, tag="wp")
            nc.scalar.dma_start(wp_sb[:], wp_d[:].rearrange("(o p) m -> p o m", p=128))
            cosr = p1.tile([128, T], BF16, tag="cosr")
            nc.scalar.dma_start(cosr[:], cosr_d[:])
            sinr = p1.tile([128, T], BF16, tag="sinr")
            nc.scalar.dma_start(sinr[:], sinr_d[:])
            esel_sb = p1.tile([128, 2], BF16, tag="esel")
            nc.scalar.dma_start(esel_sb[:], esel_d[:])
            strictu = p1.tile([128, 128], BF16, tag="strictu")
            nc.scalar.dma_start(strictu[:], strictu_d[:])
            lsel_sb = p1.tile([2, 2, 65], F32R, tag="lsel")
            nc.scalar.dma_start(lsel_sb[:], lsel_d[:].bitcast(F32R))

            A_q = p1.tile([128, 4, T], BF16, tag="A_q")
            A_k = p1.tile([128, 4, T], BF16, tag="A_k")
            qkp = p1.tile([128, 4, T], BF16, tag="qkp")
            v_sb = p1.tile([128, T // 128, HPC, 65], BF16, tag="v_sb")
            outT = p1.tile([128, 4, T], BF16, tag="outT")
            # ones column for the denominator row of the V-matmul
            nc.gpsimd.memset(v_sb[:, :, :, 64], 1.0)

            def qkv_units(tci):
                """Generator: emit QKV for token chunk tci in PE-sized units."""
                tok0 = tci * QC
                tcs = ds(tok0, QC)
                xt = pxt.tile([128, FC, QC], BF16, tag="xt", name=f"xt{tci}")
                for fc in range(FC):
                    nc.sync.dma_start(xt[:, fc, :], xT_d[ts(fc, 128), tcs])
                for w_sb, dst in ((wq_sb, A_q), (wk_sb, A_k)):
                    for ti in range(4):
                        pq = ps.tile([128, QC], F32, tag="pst", bufs=5,
                                     name="pq")
                        for fc in range(FC):
                            nc.tensor.matmul(
                                pq[:],
                                w_sb[:, fc, ts(ti, 128)],
                                xt[:, fc, :],
                                start=(fc == 0),
                                stop=(fc == FC - 1),
                            )
                            if fc == 3:
                                yield
                        nc.vector.tensor_copy(dst[:, ti, tcs], pq[:])
                        yield
                for t4 in range(4):
                    pv = ps.tile([128, QC], F32, tag="pst", bufs=5, name="pv")
                    for fc in range(FC):
                        nc.tensor.matmul(
                            pv[:],
                            xt[:, fc, ts(t4, 128)],
                            wv_sb[:, fc, :],
                            start=(fc == 0),
                            stop=(fc == FC - 1),
                        )
                        if fc == 3:
                            yield
                    nc.vector.tensor_copy(
                        v_sb[:, 4 * tci + t4, :, 0:64],
                        pv[:].rearrange("p (h d) -> p h d", h=HPC),
                    )
                    yield
                # RoPE: A = A*cos + swap(A)*sin  (sign baked into sinr)
                for A in (A_q, A_k):
                    Bt = pbt.tile([128, 4, QC], BF16, tag="Bt")
                    nc.sync.dma_start(Bt[0:32], A[32:64, :, tcs])
                    nc.sync.dma_start(Bt[32:64], A[0:32, :, tcs])
                    nc.sync.dma_start(Bt[64:96], A[96:128, :, tcs])
                    nc.sync.dma_start(Bt[96:128], A[64:96, :, tcs])
                    for ti in range(4):
                        nc.gpsimd.tensor_tensor(
                            Bt[:, ti, :], Bt[:, ti, :], sinr[:, tcs], ALU.mult
                        )
                        nc.vector.tensor_tensor(
                            A[:, ti, tcs], A[:, ti, tcs], cosr[:, tcs], ALU.mult
                        )
                        nc.vector.tensor_tensor(
                            A[:, ti, tcs], A[:, ti, tcs], Bt[:, ti, :], ALU.add
                        )
                    yield
                for ti in range(4):
                    nc.vector.tensor_tensor(
                        qkp[:, ti, tcs], A_q[:, ti, tcs], A_k[:, ti, tcs],
                        ALU.mult,
                    )
                yield

            def proj_units(qj):
                """Generator: project 512 tokens of q-chunk qj to DRAM."""
                for nt in range(4):
                    mt0 = qj * QC + nt * 128
                    for nj in range(2):
                        pp = ps.tile([128, 512], F32, tag="pst", bufs=5,
                                     name="pp")
                        for fi in range(4):
                            nc.tensor.matmul(
                                pp[:],
                                outT[:, fi, ds(mt0, 128)],
                                wp_sb[:, fi, ts(nj, 512)],
                                start=(fi == 0),
                                stop=(fi == 3),
                            )
                        po_s = psc.tile([128, 512], F32, tag="po_sb")
                        nc.vector.tensor_copy(po_s[:], pp[:])
                        nc.sync.dma_start(
                            out_d[ds(mt0, 128), ts(nj, 512)], po_s[:]
                        )
                        yield

            def emit_attention(qj, fillers):
                q0 = qj * QC
                nkc = 4 * qj + 4
                for p in range(4):
                    po = [
                        ps.tile([65, QC], F32, tag="po", bufs=2, name=f"po{h}")
                        for h in range(2)
                    ]
                    pending = None
                    for kc in range(nkc):
                        o = kc - 4 * qj
                        c0 = 128 * o if o > 0 else 0
                        pts = []
                        for h in range(2):
                            r0 = 64 * h
                            pst = ps.tile([128, QC], F32, tag="pst", bufs=5,
                                          name=f"pst{h}")
                            nc.tensor.matmul(
                                pst[:, c0:QC],
                                A_k[r0 : r0 + 64, p, ts(kc, 128)],
                                A_q[r0 : r0 + 64, p, ds(q0 + c0, QC - c0)],
                                start=True,
                                stop=True,
                            )
                            pt = ppt.tile([128, QC], BF16, tag="pt",
                                          name=f"pt{h}")
                            nc.scalar.activation(
                                pt[:, c0:QC], pst[:, c0:QC], AF.Exp, scale=0.125
                            )
                            if o >= 0:
                                nc.vector.tensor_tensor(
                                    pt[:, ds(c0, 128)],
                                    pt[:, ds(c0, 128)],
                                    strictu[:],
                                    ALU.mult,
                                )
                            pts.append(pt)
                        next(fillers, None)
                        if pending is not None:
                            pkc, ppts, pc0 = pending
                            for h in range(2):
                                nc.tensor.matmul(
                                    po[h][:, pc0:QC],
                                    v_sb[:, pkc, 2 * p + h, :],
                                    ppts[h][:, pc0:QC],
                                    start=(pkc == 0),
                                    stop=False,
                                )
                        pending = (kc, pts, c0)
                    pkc, ppts, pc0 = pending
                    for h in range(2):
                        nc.tensor.matmul(
                            po[h][:, pc0:QC],
                            v_sb[:, pkc, 2 * p + h, :],
                            ppts[h][:, pc0:QC],
                            start=(pkc == 0),
                            stop=False,
                        )
                    # XSA diag: exp(q.k/8) into the denominator row
                    pd_t = ps.tile([2, QC], F32, tag="pd", bufs=1, name="pd")
                    nc.tensor.matmul(
                        pd_t[:], esel_sb[:], qkp[:, p, ds(q0, QC)],
                        start=True, stop=True,
                    )
                    de_t = psc.tile([2, QC], F32R, tag="de")
                    nc.scalar.activation(de_t[:], pd_t[:], AF.Exp, scale=0.125)
                    for h in range(2):
                        nc.tensor.matmul(
                            po[h][:], lsel_sb[:, h, :], de_t[:],
                            start=False, stop=True,
                        )
                    for h in range(2):
                        # custom-DVE recip misreads PSUM on HW: stage via SBUF
                        dnr = psc.tile([1, QC], F32, tag="dnr")
                        nc.vector.tensor_copy(dnr[:], po[h][64:65, :])
                        rec = psc.tile([1, QC], F32, tag="rec")
                        nc.vector.reciprocal_approx_fast(rec[:], dnr[:])
                        bc = psc.tile([64, QC], F32, tag="bc")
                        nc.gpsimd.partition_broadcast(bc[:], rec[:])
                        nc.vector.tensor_tensor(
                            outT[ds(64 * h, 64), p, ds(q0, QC)],
                            po[h][0:64, :],
                            bc[:],
                            ALU.mult,
                        )

            def chain_rr(gens):
                active = list(gens)
                while active:
                    keep = []
                    for g in active:
                        try:
                            next(g)
                            keep.append(g)
                            yield
                        except StopIteration:
                            pass
                    active = keep

            # prologue: QKV for token chunk 0
            for _ in qkv_units(0):
                pass
            for qj in range(NQJ):
                gens = []
                if qj + 1 < NQJ:
                    gens.append(qkv_units(qj + 1))
                if qj >= 1:
                    gens.append(proj_units(qj - 1))
                fillers = chain_rr(gens)
                emit_attention(qj, fillers)
                for _ in fillers:
                    pass
            for _ in proj_units(NQJ - 1):
                pass

    nc.finalize()
    return nc


def _host_inputs(x, cos, sin, W_qkv, W_proj):
    """Build per-core input maps. Core c = batch (c//2), head-group (c%2)."""
    import ml_dtypes

    bf16 = ml_dtypes.bfloat16
    x = np.asarray(x, dtype=np.float32)
    cos = np.asarray(cos, dtype=np.float32)
    sin = np.asarray(sin, dtype=np.float32)
    W_qkv = np.asarray(W_qkv, dtype=np.float32)
    W_proj = np.asarray(W_proj, dtype=np.float32)

    cosT = np.ascontiguousarray(cos[0, 0].T)  # [32, T]
    sinT = np.ascontiguousarray(sin[0, 0].T)
    cosr = np.tile(cosT, (4, 1)).astype(bf16)  # [128, T]
    sinr = np.concatenate([-sinT, sinT, -sinT, sinT], axis=0).astype(bf16)

    esel = np.zeros((128, 2), np.float32)
    esel[0:64, 0] = 1.0
    esel[64:128, 1] = 1.0
    esel = esel.astype(bf16)
    lsel = np.zeros((2, 2, 65), np.float32)
    lsel[0, 0, 64] = 1.0
    lsel[1, 1, 64] = 1.0
    strictu = np.triu(np.ones((128, 128), np.float32), 1).astype(bf16)

    # per head-group weight slices
    wslices = []
    for g in range(2):
        hs = range(8 * g, 8 * g + 8)
        wq = np.concatenate([W_qkv[64 * h : 64 * h + 64] for h in hs], axis=0)
        wk = np.concatenate(
            [W_qkv[D + 64 * h : D + 64 * h + 64] for h in hs], axis=0
        )
        wv = np.concatenate(
            [W_qkv[2 * D + 64 * h : 2 * D + 64 * h + 64] for h in hs], axis=0
        )
        wqT = np.ascontiguousarray(wq.T).astype(bf16)  # [D, 512]
        wkT = np.ascontiguousarray(wk.T).astype(bf16)
        wvT = np.ascontiguousarray(wv.T).astype(bf16)
        wpT = np.ascontiguousarray(
            W_proj[:, 512 * g : 512 * g + 512].T
        ).astype(bf16)  # [512, D]
        wslices.append((wqT, wkT, wvT, wpT))

    in_maps = []
    for c in range(NCORES):
        b, g = c // 2, c % 2
        xT = np.ascontiguousarray(x[b].T).astype(bf16)  # [D, T]
        wqT, wkT, wvT, wpT = wslices[g]
        in_maps.append(
            {
                "xT": xT,
                "wqT": wqT,
                "wkT": wkT,
                "wvT": wvT,
                "wpT": wpT,
                "cosr": cosr,
                "sinr": sinr,
                "esel": esel,
                "strictu": strictu,
                "lsel": lsel,
            }
        )
    return in_maps


_NC_CACHE = {}


def _get_nc():
    if "nc" not in _NC_CACHE:
        _NC_CACHE["nc"] = _build()
    return _NC_CACHE["nc"]


def kernel(x, cos, sin, W_qkv, W_proj, _trace=False, _trace_cores=None):
    from concourse import bass_utils

    nc = _get_nc()
    in_maps = _host_inputs(x, cos, sin, W_qkv, W_proj)
    res = bass_utils.run_bass_kernel_spmd(
        nc,
        in_maps,
        core_ids=list(range(NCORES)),
        trace=_trace,
        trace_cores=_trace_cores,
    )
    out = np.zeros((B, T, D), np.float32)
    for c, r in enumerate(res.results):
        out[c // 2] += r["outp"]
    kernel.last_results = res
    return out
